# revision 3
# baseline (speedup 1.0000x reference)
"""GCNNet (SimpleConv sum-aggr + global_mean_pool + 2-layer MLP) on 8 trn2 cores.

Math: out[g] = MLP(relu(sums[g] / max(counts[g],1)))
  sums[g,:]  = sum_e w_e * x[src_e,:] * [batch[dst_e]==g]
  counts[g]  = #{i : batch[i]==g}

Sharding (v2, src-parallel): core k owns src rows [6250k, 6250(k+1)).  The
per-core sparse cell matrix A_k[src, g] (coalesced edge weights) is laid out as
49 dense windows [128, 512] quantized to uint8 with one global scale s; the
device casts u8->fp16 during the (SWDGE) DMA and accumulates
acc[96, 512] += x_w^T @ C_w over windows in f32 PSUM.  Node counts for the
core's own graph range come from 0/1-layer matrices carrying 1/s so the
epilogue's reciprocal absorbs the quantization scale.  Partial acc+counts
[97, 512] are summed across the 8 cores with an on-device collective, then
every core runs the tiny-MLP epilogue.
"""

import numpy as np

N_NODES = 50000
N_EDGES = 800000
D_FEAT = 96
D_HID = 10
N_GRAPHS = 512
CORES = 8
RPC = N_NODES // CORES          # 6250 src rows per core
GPC = N_GRAPHS // CORES         # 64 graphs per core
P = 128
NW = (RPC + P - 1) // P         # 49 windows
G = N_GRAPHS

# "AR": AllReduce [97,512], every core runs the full epilogue, host takes core0.
# "RS": ReduceScatter, each core gets its 64-graph slab, host concats.
COLLECTIVE = "AR"
N_CAST_CHUNKS = 7               # cu cast-DMA pipeline chunks

_nc_cache = {}


def _build_nc(n_layers, collective):
    import concourse.mybir as mybir
    import concourse.tile as tile
    from concourse import bacc

    f32 = mybir.dt.float32
    f16 = mybir.dt.float16
    u8 = mybir.dt.uint8
    L = n_layers

    nc = bacc.Bacc(
        "TRN2",
        target_bir_lowering=False,
        debug=False,
        num_devices=CORES,
    )

    cu_d = nc.dram_tensor("cu", [P, NW * G], u8, kind="ExternalInput")
    xw_d = nc.dram_tensor("xw", [P, NW * D_FEAT], f16, kind="ExternalInput")
    cm_d = nc.dram_tensor("cm", [P, L * G], f16, kind="ExternalInput")
    w1_d = nc.dram_tensor("w1", [D_FEAT, D_HID], f32, kind="ExternalInput")
    b1_d = nc.dram_tensor("b1", [D_HID, 1], f32, kind="ExternalInput")
    w2_d = nc.dram_tensor("w2", [D_HID, 1], f32, kind="ExternalInput")
    b2_d = nc.dram_tensor("b2", [1, 1], f32, kind="ExternalInput")
    GOUT = G if collective == "AR" else GPC
    out_d = nc.dram_tensor("out", [1, GOUT], f32, kind="ExternalOutput")

    # window chunks for the cast-DMA pipeline (front-loaded small chunks)
    sizes = []
    w = 0
    ramp = [2, 3, 5, 7]
    i = 0
    while w < NW:
        n = min(ramp[i] if i < len(ramp) else 10, NW - w)
        sizes.append(n)
        w += n
        i += 1

    with tile.TileContext(nc) as tc:
        with (
            tc.tile_pool(name="const", bufs=1) as cp,
            tc.tile_pool(name="cw", bufs=len(sizes)) as cwp,
            tc.tile_pool(name="psum", bufs=1, space="PSUM") as pp,
            tc.tile_pool(name="dram", bufs=1, space="DRAM") as dram,
        ):
            acc_ps = pp.tile([D_FEAT, G], f32, tag="acc")
            cnt_ps = pp.tile([1, G], f32, tag="cnt")

            # x windows: two DMAs so the first windows arrive early
            xw_t = cp.tile([P, NW * D_FEAT], f16, tag="xw")
            nxh = 8
            nc.sync.dma_start(
                out=xw_t[:, : nxh * D_FEAT], in_=xw_d[:, : nxh * D_FEAT]
            )
            nc.sync.dma_start(
                out=xw_t[:, nxh * D_FEAT :], in_=xw_d[:, nxh * D_FEAT :]
            )

            ones_t = cp.tile([P, 1], f16, tag="ones")
            nc.vector.memset(ones_t[:], 1.0)
            ones10_t = cp.tile([1, D_HID], f32, tag="ones10")
            nc.vector.memset(ones10_t[:], 1.0)

            cm_t = cp.tile([P, L * G], f16, tag="cm")
            nc.sync.dma_start(out=cm_t[:], in_=cm_d[:, :])
            w1_t = cp.tile([D_FEAT, D_HID], f32, tag="w1")
            nc.sync.dma_start(out=w1_t[:], in_=w1_d[:, :])
            b1_t = cp.tile([D_HID, 1], f32, tag="b1")
            nc.sync.dma_start(out=b1_t[:], in_=b1_d[:, :])
            w2_t = cp.tile([D_HID, 1], f32, tag="w2")
            nc.sync.dma_start(out=w2_t[:], in_=w2_d[:, :])
            b2_t = cp.tile([1, 1], f32, tag="b2")
            nc.sync.dma_start(out=b2_t[:], in_=b2_d[:, :])

            # main loop: cast-DMA chunk of C windows, then matmul-accumulate
            w0 = 0
            for ci, ncw in enumerate(sizes):
                cw_t = cwp.tile([P, 10 * G], f16, tag="cwt")
                nc.gpsimd.dma_start(
                    out=cw_t[:, : ncw * G],
                    in_=cu_d[:, w0 * G : (w0 + ncw) * G],
                )
                for lw in range(ncw):
                    w = w0 + lw
                    nc.tensor.matmul(
                        acc_ps[:, :],
                        lhsT=xw_t[:, w * D_FEAT : (w + 1) * D_FEAT],
                        rhs=cw_t[:, lw * G : (lw + 1) * G],
                        start=(w == 0),
                        stop=(w == NW - 1),
                    )
                w0 += ncw

            # node counts (value 1/s baked into cm)
            for l in range(L):
                nc.tensor.matmul(
                    cnt_ps[:, :],
                    lhsT=ones_t[:],
                    rhs=cm_t[:, l * G : (l + 1) * G],
                    start=(l == 0),
                    stop=(l == L - 1),
                )

            # stage partials [97, 512] and run the collective through DRAM
            part_sb = cp.tile([D_FEAT + 1, G], f32, tag="part")
            nc.vector.tensor_copy(out=part_sb[:D_FEAT, :], in_=acc_ps[:, :])
            nc.vector.tensor_copy(
                out=part_sb[D_FEAT : D_FEAT + 1, :], in_=cnt_ps[:, :]
            )

            if collective == "AR":
                ar_in = dram.tile([D_FEAT + 1, G], f32, tag="ar_in")
                ar_out = dram.tile(
                    [D_FEAT + 1, G], f32, tag="ar_out", addr_space="Shared"
                )
                nc.sync.dma_start(out=ar_in[:], in_=part_sb[:])
                nc.gpsimd.collective_compute(
                    "AllReduce",
                    mybir.AluOpType.add,
                    replica_groups=[list(range(CORES))],
                    ins=[ar_in[:].opt()],
                    outs=[ar_out[:].opt()],
                )
                all_sb = cp.tile([D_FEAT + 1, G], f32, tag="all")
                nc.sync.dma_start(out=all_sb[:], in_=ar_out[:])
            else:
                CB = (D_FEAT + 1) * GPC
                rs_in = dram.tile([1, CORES * CB], f32, tag="rs_in")
                rs_out = dram.tile([1, CB], f32, tag="rs_out", addr_space="Shared")
                # slab-major staging: slab k holds part_sb[:, 64k:64(k+1)]
                nc.sync.dma_start(
                    out=rs_in[:]
                    .reshape([CORES, D_FEAT + 1, GPC])
                    .transpose([1, 0, 2]),
                    in_=part_sb[:].reshape([D_FEAT + 1, CORES, GPC]),
                )
                nc.gpsimd.collective_compute(
                    "ReduceScatter",
                    mybir.AluOpType.add,
                    replica_groups=[list(range(CORES))],
                    ins=[rs_in[:].opt()],
                    outs=[rs_out[:].opt()],
                )
                all_sb = cp.tile([D_FEAT + 1, GPC], f32, tag="all")
                nc.sync.dma_start(
                    out=all_sb[:], in_=rs_out[:].reshape([D_FEAT + 1, GPC])
                )

            # epilogue: relu commutes with the positive 1/(s*count) scale
            GE = GOUT
            a_sb = cp.tile([D_FEAT, GE], f32, tag="a")
            nc.vector.tensor_scalar_max(a_sb[:], all_sb[:D_FEAT, :], 0.0)
            cmax = cp.tile([1, GE], f32, tag="cmax")
            nc.vector.tensor_scalar_max(
                cmax[:], all_sb[D_FEAT : D_FEAT + 1, :], 1.0
            )
            recip = cp.tile([1, GE], f32, tag="recip")
            nc.vector.reciprocal(recip[:], cmax[:])

            b_ps = pp.tile([D_HID, GE], f32, tag="b")
            nc.tensor.matmul(b_ps[:, :], lhsT=w1_t[:], rhs=a_sb[:], start=True, stop=True)
            rb_ps = pp.tile([D_HID, GE], f32, tag="rb")
            nc.tensor.matmul(
                rb_ps[:, :], lhsT=ones10_t[:], rhs=recip[:], start=True, stop=True
            )
            rb_sb = cp.tile([D_HID, GE], f32, tag="rbs")
            nc.vector.tensor_copy(out=rb_sb[:, :], in_=rb_ps[:, :])

            z_sb = cp.tile([D_HID, GE], f32, tag="z")
            nc.vector.tensor_tensor(
                z_sb[:], b_ps[:, :], rb_sb[:], mybir.AluOpType.mult
            )
            nc.vector.tensor_scalar(
                out=z_sb[:],
                in0=z_sb[:],
                scalar1=b1_t[:],
                scalar2=0.0,
                op0=mybir.AluOpType.add,
                op1=mybir.AluOpType.max,
            )

            o_ps = pp.tile([1, GE], f32, tag="o")
            nc.tensor.matmul(o_ps[:, :], lhsT=w2_t[:], rhs=z_sb[:], start=True, stop=True)
            o_sb = cp.tile([1, GE], f32, tag="os")
            nc.vector.tensor_scalar(
                out=o_sb[:],
                in0=o_ps[:, :],
                scalar1=b2_t[:],
                scalar2=None,
                op0=mybir.AluOpType.add,
            )
            nc.sync.dma_start(out=out_d[:, :], in_=o_sb[:])

    nc.compile()
    return nc


def _occurrence_ranks(key):
    """rank of each element within its equal-key group (0-based), stable."""
    order = np.argsort(key, kind="stable")
    sk = key[order]
    n = len(sk)
    if n == 0:
        return np.zeros(0, np.int64)
    starts = np.r_[0, np.flatnonzero(np.diff(sk)) + 1]
    lens = np.diff(np.r_[starts, n])
    ranks_sorted = np.arange(n) - np.repeat(starts, lens)
    ranks = np.empty(n, np.int64)
    ranks[order] = ranks_sorted
    return ranks


def prepare_inputs(x, edge_index, edge_attr, batch, W1, b1, W2, b2):
    """Host-side reformatting (placement + sparse canonicalization only)."""
    x = np.asarray(x, np.float32)
    src = np.asarray(edge_index[0], np.int64)
    dst = np.asarray(edge_index[1], np.int64)
    w = np.asarray(edge_attr, np.float32)
    batch = np.asarray(batch, np.int64)
    g = batch[dst]

    core = src // RPC
    per_core = []
    for k in range(CORES):
        m = core == k
        r = src[m] - k * RPC
        gg = g[m]
        cell_key = r * G + gg
        uniq, inv = np.unique(cell_key, return_inverse=True)
        w_cell = np.bincount(inv, weights=w[m].astype(np.float64)).astype(np.float32)
        per_core.append((uniq, w_cell))

    # count layers (core k counts nodes of its own graph range, columns global)
    node_bounds = np.searchsorted(batch, np.arange(CORES + 1) * GPC)
    ranks_all, n_layers = [], 1
    for k in range(CORES):
        n0, n1 = node_bounds[k], node_bounds[k + 1]
        gl = batch[n0:n1]
        pk = np.arange(n1 - n0) % P
        ranks = _occurrence_ranks(pk * G + gl)
        ranks_all.append((pk, ranks, gl))
        n_layers = max(n_layers, int(ranks.max(initial=-1)) + 1)

    in_maps = []
    for k in range(CORES):
        uniq, w_cell = per_core[k]
        r_c = uniq // G
        g_c = uniq % G
        # per-src-row u8 quantization; the row scale is folded into the x row
        s_row = np.zeros(RPC, np.float32)
        np.maximum.at(s_row, r_c, w_cell)
        s_row = np.where(s_row > 0, s_row, 1.0) / 255.0
        u = np.clip(np.rint(w_cell / s_row[r_c]), 0, 255).astype(np.uint8)
        cu = np.zeros((P, NW * G), np.uint8)
        cu[r_c % P, (r_c // P) * G + g_c] = u

        xk = np.zeros((NW * P, D_FEAT), np.float16)
        xk[:RPC] = (
            x[k * RPC : (k + 1) * RPC] * s_row[:, None]
        ).astype(np.float16)
        xw = np.ascontiguousarray(
            xk.reshape(NW, P, D_FEAT).transpose(1, 0, 2)
        ).reshape(P, NW * D_FEAT)

        pk, ranks, gl = ranks_all[k]
        cm = np.zeros((P, n_layers * G), np.float16)
        cm[pk, ranks * G + gl] = np.float16(1.0)

        in_maps.append(
            {
                "cu": cu,
                "xw": xw,
                "cm": cm,
                "w1": np.asarray(W1, np.float32).reshape(D_FEAT, D_HID),
                "b1": np.asarray(b1, np.float32).reshape(D_HID, 1),
                "w2": np.asarray(W2, np.float32).reshape(D_HID, 1),
                "b2": np.asarray(b2, np.float32).reshape(1, 1),
            }
        )
    return in_maps, n_layers


def get_nc(n_layers, collective=None):
    collective = collective or COLLECTIVE
    key = (n_layers, collective)
    if key not in _nc_cache:
        _nc_cache[key] = _build_nc(n_layers, collective)
    return _nc_cache[key]


def assemble(res, collective=None):
    collective = collective or COLLECTIVE
    if collective == "AR":
        out = np.asarray(res.results[0]["out"], np.float32).reshape(N_GRAPHS)
    else:
        out = np.concatenate(
            [
                np.asarray(res.results[k]["out"], np.float32).reshape(GPC)
                for k in range(CORES)
            ]
        )
    return out.reshape(N_GRAPHS, 1)


def kernel(**inputs):
    from concourse import bass_utils

    in_maps, n_layers = prepare_inputs(**inputs)
    nc = get_nc(n_layers)
    res = bass_utils.run_bass_kernel_spmd(nc, in_maps, core_ids=list(range(CORES)))
    return assemble(res)
